# revision 10
# baseline (speedup 1.0000x reference)
"""CTC loss (tf.keras ctc_batch_cost semantics) on 8 Trainium2 NeuronCores.

Sharding: data-parallel over batch -- each of the 8 cores runs the CTC DP
for 32 examples (the DP is independent per example); the runner hands each
core its axis-0 slice of the inputs and concatenates the per-core [32, 1]
losses.

Math: the CTC forward runs in *linear* probability space with a constant
per-step boost  p~ = K * (y_pred + eps), K = e^0.15.  Every path through
the T=512 trellis picks up exactly T boost factors, so
loss = -(ln(alpha_T[S-1] + alpha_T[S-2]) - T*ln K).  K is tuned so the
whole trellis stays inside fp32 range on these inputs (peak ~5e34);
values that underflow to zero correspond to paths ~e^-90 below the
dominant ones -- numerically irrelevant, the same role the -1e30 "NEG"
plays in the reference's log-space DP.

The recurrence splits into even (blank) and odd (label) lanes:
    E[j,t] = pb[t] * (E[j,t-1] + O[j-1,t-1])                       (s = 2j)
    O[j,t] = pl[j,t] * (O[j,t-1] + E[j,t-1] + sk[j]*O[j-1,t-1])    (s = 2j+1)
Each lane is a first-order linear recurrence along t, which maps to ONE
DVE `tensor_tensor_scan` instruction (state = d0*state + d1) covering all
512 time steps -- the sequential dimension collapses from T=512 elementwise
steps (the reference's scan) to 65 lane sweeps of <=5 wide vector ops.
The DP runs in fp32; end-to-end error vs the fp32 log-space reference is
~1.5e-4 (from the fp16 rounding of p~, verified on HW).

The DP only ever reads y_pred at the 65 extended-label columns of each
example (64 labels + blank), and the label set is constant over t, so the
column gather is done on the host (a cheap take_along_axis) and only the
gathered probabilities [B, 65, T] are shipped to the devices, linearly
quantized to uint8 -- 8.2 MiB instead of the 128 MiB raw y_pred (the
wall clock is dominated by the ~55 MB/s axon tunnel, so bytes ~= time).
The device dequantizes with p~ = (K/255)*u + K*eps on the scalar engine
and runs the 65-lane scan DP.  uint8 quantization of y adds ~1.7e-3 max
rel err on the loss (verified in numpy sim and on HW; fp16 shipping gives
1.1e-4 at 2x the bytes).

Dispatch: the jitted SPMD callable (same _bass_exec_p custom-call path
run_bass_kernel_spmd uses under axon) is built once and cached at module
level, so repeat calls reuse the loaded executable instead of re-tracing,
re-compiling and re-loading it on every invocation.
"""
import numpy as np

import concourse.bass as bass
import concourse.bacc as bacc
import concourse.tile as tile
from concourse import mybir

B, T, C, L = 256, 512, 256, 64
NCORES = 8
BC = B // NCORES
NL = L + 1
EPS = 1e-7
CBOOST = 0.15
KF = float(np.float16(np.exp(CBOOST)))     # fp16-representable boost
CB_EFF = float(np.log(KF))

F32 = mybir.dt.float32
F16 = mybir.dt.float16
U8 = mybir.dt.uint8


def _emit(nc, tc, pl_d, sks, loss):
    with tc.tile_pool(name="dp", bufs=1) as dp:
        skt = dp.tile([BC, L], F32, name="skt")
        nc.sync.dma_start(out=skt[:], in_=sks[:])
        plq = dp.tile([BC, NL * T], U8, name="plq")
        nc.sync.dma_start(out=plq[:], in_=pl_d[:])
        # dequant: p~ = (K/255)*u + K*eps
        pl = dp.tile([BC, NL * T], F16, name="pl")
        nc.scalar.activation(
            out=pl[:], in_=plq[:], func=mybir.ActivationFunctionType.Copy,
            scale=KF / 255.0, bias=KF * EPS)

        # ---- DP over 65 lane pairs ----
        zz = dp.tile([BC, T], F32, name="zz")
        d1e = dp.tile([BC, T], F32, name="d1e")
        uu = dp.tile([BC, T], F32, name="uu")
        d1o = dp.tile([BC, T], F32, name="d1o")
        ee = dp.tile([BC, T], F32, name="ee")
        oa = dp.tile([BC, T], F32, name="oa")
        ob = dp.tile([BC, T], F32, name="ob")
        nc.vector.memset(zz[:], 0.0)
        nc.vector.memset(d1e[:], 0.0)
        nc.vector.memset(uu[:], 0.0)
        nc.vector.memset(d1o[:], 0.0)

        pb = pl[:, L * T:(L + 1) * T]
        mlt, pls = mybir.AluOpType.mult, mybir.AluOpType.add

        o_prev = zz
        for j in range(NL):
            # lane-j tail truncation: E[j] past t=447+j (O[j] past 448+j)
            # cannot reach s >= S-2 by t=T-1, so skip computing it
            TE = min(449 + j, T)
            TO = min(450 + j, T)
            if j == 0:
                nc.vector.tensor_tensor_scan(
                    ee[:, 0:TE], pb[:, 0:TE], zz[:, 0:TE], 1.0, mlt, pls)
            else:
                nc.vector.tensor_tensor(
                    out=d1e[:, 1:TE], in0=pb[:, 1:TE],
                    in1=o_prev[:, 0:TE - 1], op=mlt)
                nc.vector.tensor_tensor_scan(
                    ee[:, 0:TE], pb[:, 0:TE], d1e[:, 0:TE], 0.0, mlt, pls)
            if j < L:
                o_cur = oa if (j % 2 == 0) else ob
                plj = pl[:, j * T:(j + 1) * T]
                nc.vector.scalar_tensor_tensor(
                    out=uu[:, 1:TO], in0=o_prev[:, 0:TO - 1],
                    scalar=skt[:, j:j + 1], in1=ee[:, 0:TO - 1],
                    op0=mlt, op1=pls)
                nc.vector.tensor_tensor(
                    out=d1o[:, 1:TO], in0=plj[:, 1:TO], in1=uu[:, 1:TO],
                    op=mlt)
                nc.vector.tensor_tensor_scan(
                    o_cur[:, 0:TO], plj[:, 0:TO], d1o[:, 0:TO],
                    1.0 if j == 0 else 0.0, mlt, pls)
                o_prev = o_cur

        fin = dp.tile([BC, 1], F32, name="fin")
        lg = dp.tile([BC, 1], F32, name="lg")
        lo = dp.tile([BC, 1], F32, name="lo")
        nc.vector.tensor_tensor(
            out=fin[:], in0=ee[:, T - 1:T], in1=o_prev[:, T - 1:T], op=pls)
        nc.scalar.activation(
            out=lg[:], in_=fin[:], func=mybir.ActivationFunctionType.Ln)
        nc.vector.tensor_scalar(
            out=lo[:], in0=lg[:], scalar1=-1.0, scalar2=float(T) * CB_EFF,
            op0=mlt, op1=pls)
        nc.sync.dma_start(out=loss[:], in_=lo[:])


_CACHED_NC = None


def _build():
    global _CACHED_NC
    if _CACHED_NC is not None:
        return _CACHED_NC
    nc = bacc.Bacc("TRN2", target_bir_lowering=False, debug=False)
    pl_d = nc.dram_tensor("pl", [BC, NL * T], U8, kind="ExternalInput")
    sks = nc.dram_tensor("skips", [BC, L], F32, kind="ExternalInput")
    loss = nc.dram_tensor("loss", [BC, 1], F32, kind="ExternalOutput")
    with tile.TileContext(nc) as tc:
        _emit(nc, tc, pl_d, sks, loss)
    nc.compile()
    _CACHED_NC = nc
    return nc


class _CachedSpmdRunner:
    """One-time-built jitted SPMD dispatch for a compiled Bass module.

    Mirrors what bass_utils.run_bass_kernel_spmd does under axon
    (shard_map over the _bass_exec_p custom-call on jax.devices()[:n]),
    but keeps the jitted callable so warm calls skip re-trace/re-compile
    and the remote keeps the loaded executable.
    """

    def __init__(self, nc, n_cores):
        import jax
        from jax.sharding import Mesh, PartitionSpec
        try:
            from jax.experimental.shard_map import shard_map
        except ImportError:  # newer jax
            from jax import shard_map
        from concourse.bass2jax import (
            install_neuronx_cc_hook, _bass_exec_p, partition_id_tensor)

        install_neuronx_cc_hook()
        self.n_cores = n_cores
        partition_name = (nc.partition_id_tensor.name
                          if nc.partition_id_tensor else None)
        in_names, out_names, out_avals, zero_outs = [], [], [], []
        for alloc in nc.m.functions[0].allocations:
            if not isinstance(alloc, mybir.MemoryLocationSet):
                continue
            name = alloc.memorylocations[0].name
            if alloc.kind == "ExternalInput":
                if name != partition_name:
                    in_names.append(name)
            elif alloc.kind == "ExternalOutput":
                shape = tuple(alloc.tensor_shape)
                dtype = mybir.dt.np(alloc.dtype)
                out_avals.append(jax.core.ShapedArray(shape, dtype))
                out_names.append(name)
                zero_outs.append(np.zeros(shape, dtype))
        self.dbg_name = None
        if nc.dbg_addr is not None:
            if nc.dbg_callbacks:
                raise RuntimeError("dbg_callbacks unsupported in this runner")
            self.dbg_name = nc.dbg_addr.name
            if self.dbg_name in in_names:
                in_names.remove(self.dbg_name)
            in_names.append(self.dbg_name)
        self.in_names = in_names
        self.out_names = out_names
        self.zero_outs = zero_outs
        n_params = len(in_names)
        n_outs = len(out_avals)
        all_in_names = in_names + out_names + (
            [partition_name] if partition_name else [])

        def _body(*args):
            operands = list(args)
            if partition_name is not None:
                operands.append(partition_id_tensor())
            outs = _bass_exec_p.bind(
                *operands,
                out_avals=tuple(out_avals),
                in_names=tuple(all_in_names),
                out_names=tuple(out_names),
                lowering_input_output_aliases=(),
                sim_require_finite=True,
                sim_require_nnan=True,
                nc=nc,
            )
            return tuple(outs)

        devices = jax.devices()[:n_cores]
        assert len(devices) == n_cores
        mesh = Mesh(np.asarray(devices), ("core",))
        in_specs = (PartitionSpec("core"),) * (n_params + n_outs)
        out_specs = (PartitionSpec("core"),) * n_outs
        donate = tuple(range(n_params, n_params + n_outs))
        self.fn = jax.jit(
            shard_map(_body, mesh=mesh, in_specs=in_specs,
                      out_specs=out_specs, check_rep=False),
            donate_argnums=donate, keep_unused=True,
        )

    def run(self, in_map):
        """in_map: full (n_cores*per_core_rows, ...) arrays keyed by name."""
        ins = []
        for name in self.in_names:
            if name == self.dbg_name:
                ins.append(np.zeros((self.n_cores, 2), np.uint32))
            else:
                ins.append(np.ascontiguousarray(in_map[name]))
        zeros = [np.zeros((self.n_cores * z.shape[0], *z.shape[1:]), z.dtype)
                 for z in self.zero_outs]
        out_arrs = self.fn(*ins, *zeros)
        return {name: np.asarray(a)
                for name, a in zip(self.out_names, out_arrs)}


_RUNNER = None


def _get_runner():
    global _RUNNER
    if _RUNNER is None:
        _RUNNER = _CachedSpmdRunner(_build(), NCORES)
    return _RUNNER


def _host_prep(y_true, y_pred):
    lab = np.asarray(y_true).astype(np.int32)
    ypf = np.asarray(y_pred)
    if ypf.dtype != np.float32:
        ypf = ypf.astype(np.float32)
    cols = np.concatenate(
        [lab, np.full((B, 1), C - 1, np.int32)], axis=1)        # [B, NL]
    # uint8 linear quantization of y (dequantized on device as
    # (K/255)*u + K*eps); verified max rel err ~1.7e-3 on the loss.
    pl = np.empty((B, NL, T), np.uint8)

    def _prep_slice(s):
        g = np.take_along_axis(ypf[s], cols[s][:, None, :], axis=2)
        pl[s] = (g.transpose(0, 2, 1) * np.float32(255.0)
                 + np.float32(0.5)).astype(np.uint8)

    # 8 slices even on one CPU: overlapping the gather's memory stalls
    # measures ~20% faster, and it scales when more cores are present.
    from concurrent.futures import ThreadPoolExecutor
    nth = 8
    step = (B + nth - 1) // nth
    with ThreadPoolExecutor(nth) as ex:
        list(ex.map(_prep_slice,
                    [slice(i, min(i + step, B)) for i in range(0, B, step)]))
    sks = np.zeros((B, L), np.float32)
    sks[:, 1:] = (lab[:, 1:] != lab[:, :-1]).astype(np.float32)
    return pl.reshape(B, NL * T), sks


def kernel(y_true, y_pred):
    global _RUNNER
    pl, sks = _host_prep(y_true, y_pred)
    out = None
    for attempt in range(2):
        try:
            res = _get_runner().run({"pl": pl, "skips": sks})
            out = res["loss"]
            break
        except Exception:
            # e.g. transient NRT_EXEC_UNIT_UNRECOVERABLE: rebuild the
            # jitted dispatch (fresh executable load) and retry once.
            _RUNNER = None
    if out is None:
        # Fallback: the stock per-call SPMD dispatch path.
        from concourse.bass_utils import run_bass_kernel_spmd
        nc = _build()
        in_maps = [
            {"pl": pl[c * BC:(c + 1) * BC], "skips": sks[c * BC:(c + 1) * BC]}
            for c in range(NCORES)
        ]
        r = run_bass_kernel_spmd(nc, in_maps, list(range(NCORES)))
        out = np.concatenate(
            [r.results[i]["loss"] for i in range(NCORES)], axis=0)
    return np.ascontiguousarray(out).astype(np.float32)


# revision 12
# speedup vs baseline: 1.1366x; 1.1366x over previous
"""CTC loss (tf.keras ctc_batch_cost semantics) on 8 Trainium2 NeuronCores.

Sharding: data-parallel over batch -- each of the 8 cores runs the CTC DP
for 32 examples (the DP is independent per example); the runner hands each
core its axis-0 slice of the inputs and concatenates the per-core [32, 1]
losses.

Math: the CTC forward runs in *linear* probability space with a constant
per-step boost  p~ = K * (y_pred + eps), K = e^0.15.  Every path through
the T=512 trellis picks up exactly T boost factors, so
loss = -(ln(alpha_T[S-1] + alpha_T[S-2]) - T*ln K).  K is tuned so the
whole trellis stays inside fp32 range on these inputs (peak ~5e34);
values that underflow to zero correspond to paths ~e^-90 below the
dominant ones -- numerically irrelevant, the same role the -1e30 "NEG"
plays in the reference's log-space DP.

The recurrence splits into even (blank) and odd (label) lanes:
    E[j,t] = pb[t] * (E[j,t-1] + O[j-1,t-1])                       (s = 2j)
    O[j,t] = pl[j,t] * (O[j,t-1] + E[j,t-1] + sk[j]*O[j-1,t-1])    (s = 2j+1)
Each lane is a first-order linear recurrence along t, which maps to ONE
DVE `tensor_tensor_scan` instruction (state = d0*state + d1) covering the
whole lane -- the sequential dimension collapses from T=512 elementwise
steps (the reference's scan) to 65 lane sweeps of <=5 wide vector ops.

Trellis reachability truncates every lane to a fixed window: lane j can
only matter for t in [j, j+449) (forward: s <= 2t+1; backward: s must
still reach S-2 by t=T-1), so label lanes are stored PACKED, lane j
holding exactly its 449-step window (E lanes use the first 448).  In
packed coordinates p = t - j the cross-lane time shift disappears
(O[j-1, t-1] sits at packed index p of lane j-1) while the within-lane
shift E[j, t-1] becomes p-1, handled by a zero-padded leading column.

The DP only ever reads y_pred at the 65 extended-label columns of each
example (64 labels + blank), and the label set is constant over t, so the
column gather runs on the host (a cheap take_along_axis) and only the
packed windows ship to the devices, linearly quantized to uint8 --
7.2 MiB total instead of the 128 MiB raw y_pred (the wall clock is
dominated by the ~55 MB/s axon tunnel, so shipped bytes ~= time).  The
device dequantizes with p~ = (K/255)*u + K*eps on the scalar engine and
runs the 65-lane scan DP in fp32.  uint8 quantization adds ~1.7e-3 max
rel err on the loss (verified in numpy sim and on HW; the sim of the
packed DP matches the unpacked one exactly).  Skip flags ride along as
64 trailing uint8 bytes per example.

Dispatch: the jitted SPMD callable (same _bass_exec_p custom-call path
run_bass_kernel_spmd uses under axon) is built once and cached at module
level, so repeat calls reuse the loaded executable instead of re-tracing,
re-compiling and re-loading it on every invocation.
"""
import numpy as np

import concourse.bass as bass
import concourse.bacc as bacc
import concourse.tile as tile
from concourse import mybir

B, T, C, L = 256, 512, 256, 64
NCORES = 8
BC = B // NCORES
NL = L + 1
EPS = 1e-7
CBOOST = 0.15
KF = float(np.float16(np.exp(CBOOST)))     # fp16-representable boost
CB_EFF = float(np.log(KF))

WO = 449                   # O-lane packed window (t in [j, j+449))
WE = 448                   # E-lane packed window (t in [j, j+448))
PBASE = L * WO             # 28736: start of the blank lane (full T wide)
SKBASE = PBASE + T         # 29248: start of the 64 skip-flag bytes
PACK = SKBASE + L          # 29312 bytes per example

F32 = mybir.dt.float32
F16 = mybir.dt.float16
U8 = mybir.dt.uint8


def _emit(nc, tc, plp_d, loss):
    with tc.tile_pool(name="dp", bufs=1) as dp:
        plq = dp.tile([BC, PACK], U8, name="plq")
        nc.sync.dma_start(out=plq[:], in_=plp_d[:])
        # dequant: p~ = (K/255)*u + K*eps  (label windows + blank lane)
        pf = dp.tile([BC, SKBASE], F16, name="pf")
        nc.scalar.activation(
            out=pf[:], in_=plq[:, 0:SKBASE],
            func=mybir.ActivationFunctionType.Copy,
            scale=KF / 255.0, bias=KF * EPS)
        # skip flags: exact 0.0 / 1.0
        skt = dp.tile([BC, L], F32, name="skt")
        nc.scalar.activation(
            out=skt[:], in_=plq[:, SKBASE:PACK],
            func=mybir.ActivationFunctionType.Copy, scale=1.0)
        pb = pf[:, PBASE:PBASE + T]

        # ---- DP over 65 lane pairs, packed coordinates p = t - j ----
        zz = dp.tile([BC, WO], F32, name="zz")
        d1e = dp.tile([BC, WE], F32, name="d1e")
        uu = dp.tile([BC, WO], F32, name="uu")
        d1o = dp.tile([BC, WO], F32, name="d1o")
        eex = dp.tile([BC, WO], F32, name="eex")   # col 0 stays 0 = E[j,p-1] pad
        oa = dp.tile([BC, WO], F32, name="oa")
        ob = dp.tile([BC, WO], F32, name="ob")
        nc.vector.memset(zz[:], 0.0)
        nc.vector.memset(eex[:], 0.0)

        mlt, pls = mybir.AluOpType.mult, mybir.AluOpType.add

        o_prev = zz
        for j in range(NL):
            pbj = pb[:, j:j + WE]
            if j == 0:
                nc.vector.tensor_tensor_scan(
                    eex[:, 1:WO], pbj, zz[:, 0:WE], 1.0, mlt, pls)
            else:
                nc.vector.tensor_tensor(
                    out=d1e[:], in0=pbj, in1=o_prev[:, 0:WE], op=mlt)
                nc.vector.tensor_tensor_scan(
                    eex[:, 1:WO], pbj, d1e[:], 0.0, mlt, pls)
            if j < L:
                o_cur = oa if (j % 2 == 0) else ob
                plj = pf[:, j * WO:(j + 1) * WO]
                nc.vector.scalar_tensor_tensor(
                    out=uu[:], in0=o_prev[:],
                    scalar=skt[:, j:j + 1], in1=eex[:],
                    op0=mlt, op1=pls)
                nc.vector.tensor_tensor(
                    out=d1o[:], in0=plj, in1=uu[:], op=mlt)
                nc.vector.tensor_tensor_scan(
                    o_cur[:], plj, d1o[:],
                    1.0 if j == 0 else 0.0, mlt, pls)
                o_prev = o_cur

        # loss = -(ln(E[64, T-1] + O[63, T-1]) - T*ln K)
        fin = dp.tile([BC, 1], F32, name="fin")
        lg = dp.tile([BC, 1], F32, name="lg")
        lo = dp.tile([BC, 1], F32, name="lo")
        nc.vector.tensor_tensor(
            out=fin[:], in0=eex[:, WO - 1:WO], in1=o_prev[:, WO - 1:WO],
            op=pls)
        nc.scalar.activation(
            out=lg[:], in_=fin[:], func=mybir.ActivationFunctionType.Ln)
        nc.vector.tensor_scalar(
            out=lo[:], in0=lg[:], scalar1=-1.0, scalar2=float(T) * CB_EFF,
            op0=mlt, op1=pls)
        nc.sync.dma_start(out=loss[:], in_=lo[:])


_CACHED_NC = None


def _build():
    global _CACHED_NC
    if _CACHED_NC is not None:
        return _CACHED_NC
    nc = bacc.Bacc("TRN2", target_bir_lowering=False, debug=False)
    plp_d = nc.dram_tensor("pl", [BC, PACK], U8, kind="ExternalInput")
    loss = nc.dram_tensor("loss", [BC, 1], F32, kind="ExternalOutput")
    with tile.TileContext(nc) as tc:
        _emit(nc, tc, plp_d, loss)
    nc.compile()
    _CACHED_NC = nc
    return nc


class _CachedSpmdRunner:
    """One-time-built jitted SPMD dispatch for a compiled Bass module.

    Mirrors what bass_utils.run_bass_kernel_spmd does under axon
    (shard_map over the _bass_exec_p custom-call on jax.devices()[:n]),
    but keeps the jitted callable so warm calls skip re-trace/re-compile
    and the remote keeps the loaded executable.
    """

    def __init__(self, nc, n_cores):
        import jax
        from jax.sharding import Mesh, PartitionSpec
        try:
            from jax.experimental.shard_map import shard_map
        except ImportError:  # newer jax
            from jax import shard_map
        from concourse.bass2jax import (
            install_neuronx_cc_hook, _bass_exec_p, partition_id_tensor)

        install_neuronx_cc_hook()
        self.n_cores = n_cores
        partition_name = (nc.partition_id_tensor.name
                          if nc.partition_id_tensor else None)
        in_names, out_names, out_avals, zero_outs = [], [], [], []
        for alloc in nc.m.functions[0].allocations:
            if not isinstance(alloc, mybir.MemoryLocationSet):
                continue
            name = alloc.memorylocations[0].name
            if alloc.kind == "ExternalInput":
                if name != partition_name:
                    in_names.append(name)
            elif alloc.kind == "ExternalOutput":
                shape = tuple(alloc.tensor_shape)
                dtype = mybir.dt.np(alloc.dtype)
                out_avals.append(jax.core.ShapedArray(shape, dtype))
                out_names.append(name)
                zero_outs.append(np.zeros(shape, dtype))
        self.dbg_name = None
        if nc.dbg_addr is not None:
            if nc.dbg_callbacks:
                raise RuntimeError("dbg_callbacks unsupported in this runner")
            self.dbg_name = nc.dbg_addr.name
            if self.dbg_name in in_names:
                in_names.remove(self.dbg_name)
            in_names.append(self.dbg_name)
        self.in_names = in_names
        self.out_names = out_names
        self.zero_outs = zero_outs
        n_params = len(in_names)
        n_outs = len(out_avals)
        all_in_names = in_names + out_names + (
            [partition_name] if partition_name else [])

        def _body(*args):
            operands = list(args)
            if partition_name is not None:
                operands.append(partition_id_tensor())
            outs = _bass_exec_p.bind(
                *operands,
                out_avals=tuple(out_avals),
                in_names=tuple(all_in_names),
                out_names=tuple(out_names),
                lowering_input_output_aliases=(),
                sim_require_finite=True,
                sim_require_nnan=True,
                nc=nc,
            )
            return tuple(outs)

        devices = jax.devices()[:n_cores]
        assert len(devices) == n_cores
        mesh = Mesh(np.asarray(devices), ("core",))
        in_specs = (PartitionSpec("core"),) * (n_params + n_outs)
        out_specs = (PartitionSpec("core"),) * n_outs
        donate = tuple(range(n_params, n_params + n_outs))
        self.fn = jax.jit(
            shard_map(_body, mesh=mesh, in_specs=in_specs,
                      out_specs=out_specs, check_rep=False),
            donate_argnums=donate, keep_unused=True,
        )

    def run(self, in_map):
        """in_map: full (n_cores*per_core_rows, ...) arrays keyed by name."""
        ins = []
        for name in self.in_names:
            if name == self.dbg_name:
                ins.append(np.zeros((self.n_cores, 2), np.uint32))
            else:
                ins.append(np.ascontiguousarray(in_map[name]))
        zeros = [np.zeros((self.n_cores * z.shape[0], *z.shape[1:]), z.dtype)
                 for z in self.zero_outs]
        out_arrs = self.fn(*ins, *zeros)
        return {name: np.asarray(a)
                for name, a in zip(self.out_names, out_arrs)}


_RUNNER = None


def _get_runner():
    global _RUNNER
    if _RUNNER is None:
        _RUNNER = _CachedSpmdRunner(_build(), NCORES)
    return _RUNNER


def _host_prep(y_true, y_pred):
    lab = np.asarray(y_true).astype(np.int32)
    ypf = np.asarray(y_pred)
    if ypf.dtype != np.float32:
        ypf = ypf.astype(np.float32)
    cols = np.concatenate(
        [lab, np.full((B, 1), C - 1, np.int32)], axis=1)        # [B, NL]
    # uint8 linear quantization of y (dequantized on device as
    # (K/255)*u + K*eps); verified max rel err ~1.7e-3 on the loss.
    plp = np.empty((B, PACK), np.uint8)

    def _prep_slice(s):
        g = np.take_along_axis(ypf[s], cols[s][:, None, :], axis=2)
        nb = g.shape[0]
        q = np.empty((nb, NL, T), np.uint8)
        np.copyto(q, g.transpose(0, 2, 1) * np.float32(255.0)
                  + np.float32(0.5), casting="unsafe")
        # packed label windows: lane j = q[b, j, j:j+WO]
        v = np.lib.stride_tricks.as_strided(
            q, shape=(nb, L, WO), strides=(NL * T, T + 1, 1))
        plp[s, :PBASE] = v.reshape(nb, PBASE)
        plp[s, PBASE:SKBASE] = q[:, L, :]                       # blank lane

    # 8 slices even on one CPU: overlapping the gather's memory stalls
    # measures ~20% faster, and it scales when more cores are present.
    from concurrent.futures import ThreadPoolExecutor
    nth = 8
    step = (B + nth - 1) // nth
    with ThreadPoolExecutor(nth) as ex:
        list(ex.map(_prep_slice,
                    [slice(i, min(i + step, B)) for i in range(0, B, step)]))
    plp[:, SKBASE] = 0
    plp[:, SKBASE + 1:PACK] = (lab[:, 1:] != lab[:, :-1])
    return plp


def kernel(y_true, y_pred):
    global _RUNNER
    plp = _host_prep(y_true, y_pred)
    out = None
    for attempt in range(2):
        try:
            res = _get_runner().run({"pl": plp})
            out = res["loss"]
            break
        except Exception:
            # e.g. transient NRT_EXEC_UNIT_UNRECOVERABLE: rebuild the
            # jitted dispatch (fresh executable load) and retry once.
            _RUNNER = None
    if out is None:
        # Fallback: the stock per-call SPMD dispatch path.
        from concourse.bass_utils import run_bass_kernel_spmd
        nc = _build()
        in_maps = [{"pl": plp[c * BC:(c + 1) * BC]} for c in range(NCORES)]
        r = run_bass_kernel_spmd(nc, in_maps, list(range(NCORES)))
        out = np.concatenate(
            [r.results[i]["loss"] for i in range(NCORES)], axis=0)
    return np.ascontiguousarray(out).astype(np.float32)


# revision 22
# speedup vs baseline: 1.7955x; 1.5796x over previous
"""CTC loss (tf.keras ctc_batch_cost semantics) on 8 Trainium2 NeuronCores.

Sharding: data-parallel over batch -- each of the 8 cores runs the CTC DP
for 32 examples (the DP is independent per example); the runner hands each
core its axis-0 slice of the inputs and concatenates the per-core [32, 1]
losses.

Math: the CTC forward runs in *linear* probability space with a constant
per-step boost  p~ = K * (y_pred + eps), K = e^0.15.  Every path through
the T=512 trellis picks up exactly T boost factors, so
loss = -(ln(alpha_T[S-1] + alpha_T[S-2]) - T*ln K).  K is tuned so the
whole trellis stays inside fp32 range on these inputs (peak ~5e34);
values that underflow to zero correspond to paths ~e^-90 below the
dominant ones -- numerically irrelevant, the same role the -1e30 "NEG"
plays in the reference's log-space DP.

The recurrence splits into even (blank) and odd (label) lanes:
    E[j,t] = pb[t] * (E[j,t-1] + O[j-1,t-1])                       (s = 2j)
    O[j,t] = pl[j,t] * (O[j,t-1] + E[j,t-1] + sk[j]*O[j-1,t-1])    (s = 2j+1)
Each lane is a first-order linear recurrence along t, which maps to ONE
DVE `tensor_tensor_scan` instruction (state = d0*state + d1) covering the
whole lane -- the sequential dimension collapses from T=512 elementwise
steps (the reference's scan) to 65 lane sweeps of <=5 wide vector ops.

Trellis reachability truncates every lane to a fixed window: lane j can
only matter for t in [j, j+449) (forward: s <= 2t+1; backward: s must
still reach S-2 by t=T-1), so label lanes are stored PACKED, lane j
holding exactly its 449-step window (E lanes use the first 448).  In
packed coordinates p = t - j the cross-lane time shift disappears
(O[j-1, t-1] sits at packed index p of lane j-1) while the within-lane
shift E[j, t-1] becomes p-1, handled by a zero-padded leading column.

The DP only ever reads y_pred at the 65 extended-label columns of each
example (64 labels + blank), and the label set is constant over t, so the
column gather runs on the host (a fused numba pass; numpy fallback) and
only the packed windows ship to the devices -- 4.5 MiB total instead of
the 128 MiB raw y_pred (the wall clock is dominated by the ~55 MB/s axon
tunnel, so shipped bytes ~= time).  Mixed-precision quantization: label
lanes at 5 BITS (32 levels, bit-packed 8 values -> 5 bytes), the blank
lane at 8 bits -- the blank feeds all 65 E-lane scans coherently so its
error weight is outsized and the extra 128 B/example buys a 30% error
cut.  The pack uses an eighth-plane layout (value plane i = stream
positions [i*NE,(i+1)*NE)) so the device-side unpack (DVE shift/and/or
on uint8) and the dequant activations p~ = (K/31)*v + K*eps (labels),
(K/255)*u + K*eps (blank) read and write contiguously; the DP then runs
the 65-lane scan in fp32 unchanged.  This gives 1.17e-2 max rel err on
the loss vs the 2e-2 gate -- the numpy sim of this exact pipeline has
predicted the HW error to all printed digits on every configuration
tried.  Skip flags ride along as 64 trailing bytes per example.

Dispatch: the jitted SPMD callable (same _bass_exec_p custom-call path
run_bass_kernel_spmd uses under axon) is built once and cached at module
level, so repeat calls reuse the loaded executable instead of re-tracing,
re-compiling and re-loading it on every invocation.
"""
import numpy as np

import concourse.bass as bass
import concourse.bacc as bacc
import concourse.tile as tile
from concourse import mybir

B, T, C, L = 256, 512, 256, 64
NCORES = 8
BC = B // NCORES
NL = L + 1
EPS = 1e-7
CBOOST = 0.15
KF = float(np.float16(np.exp(CBOOST)))     # fp16-representable boost
CB_EFF = float(np.log(KF))

WO = 449                   # O-lane packed window (t in [j, j+449))
WE = 448                   # E-lane packed window (t in [j, j+448))
PBASE = L * WO             # 28736: start of the blank lane (full T wide)
NV = PBASE + T             # 29248 dequantized values per example
NE = PBASE // 8            # 3592: eighth-plane size (8 values -> 5 bytes)
PB5 = 5 * NE               # 17960: start of the 8-bit blank lane bytes
SKBASE = PB5 + T           # 18472: start of the 64 skip-flag bytes
PACK = SKBASE + L          # 18536 bytes per example
QLEV = 31.0                # 5-bit label quantization levels
QBL = 255.0                # 8-bit blank quantization levels

F32 = mybir.dt.float32
F16 = mybir.dt.float16
U8 = mybir.dt.uint8


def _emit(nc, tc, plp_d, loss):
    sr = mybir.AluOpType.logical_shift_right
    sl = mybir.AluOpType.logical_shift_left
    band = mybir.AluOpType.bitwise_and
    bor = mybir.AluOpType.bitwise_or
    with tc.tile_pool(name="dp", bufs=1) as dp:
        plq = dp.tile([BC, PACK], U8, name="plq")
        nc.sync.dma_start(out=plq[:], in_=plp_d[:])
        # 5-bit label unpack, eighth-plane layout: value plane i covers
        # positions [i*NE, (i+1)*NE) of the packed-lane stream, so byte
        # reads, value writes and dequant are all contiguous.
        #   B0 = v0<<3 | v1>>2;          B1 = (v1&3)<<6 | v2<<1 | v3>>4
        #   B2 = (v3&15)<<4 | v4>>1;     B3 = (v4&1)<<7 | v5<<2 | v6>>3
        #   B4 = (v6&7)<<5 | v7
        bp = [plq[:, i * NE:(i + 1) * NE] for i in range(5)]
        uq = dp.tile([BC, PBASE], U8, name="uq")
        ta = dp.tile([BC, NE], U8, name="ta")
        tb = dp.tile([BC, NE], U8, name="tb")
        vp = [uq[:, i * NE:(i + 1) * NE] for i in range(8)]
        # v0 = B0 >> 3
        nc.vector.tensor_scalar(
            out=vp[0], in0=bp[0], scalar1=3, scalar2=None, op0=sr)
        # v1 = (B0 & 7) << 2 | B1 >> 6
        nc.vector.tensor_scalar(
            out=ta[:], in0=bp[0], scalar1=7, scalar2=2, op0=band, op1=sl)
        nc.vector.tensor_scalar(
            out=tb[:], in0=bp[1], scalar1=6, scalar2=None, op0=sr)
        nc.vector.tensor_tensor(out=vp[1], in0=ta[:], in1=tb[:], op=bor)
        # v2 = (B1 >> 1) & 31
        nc.vector.tensor_scalar(
            out=vp[2], in0=bp[1], scalar1=1, scalar2=31, op0=sr, op1=band)
        # v3 = (B1 & 1) << 4 | B2 >> 4
        nc.vector.tensor_scalar(
            out=ta[:], in0=bp[1], scalar1=1, scalar2=4, op0=band, op1=sl)
        nc.vector.tensor_scalar(
            out=tb[:], in0=bp[2], scalar1=4, scalar2=None, op0=sr)
        nc.vector.tensor_tensor(out=vp[3], in0=ta[:], in1=tb[:], op=bor)
        # v4 = (B2 & 15) << 1 | B3 >> 7
        nc.vector.tensor_scalar(
            out=ta[:], in0=bp[2], scalar1=15, scalar2=1, op0=band, op1=sl)
        nc.vector.tensor_scalar(
            out=tb[:], in0=bp[3], scalar1=7, scalar2=None, op0=sr)
        nc.vector.tensor_tensor(out=vp[4], in0=ta[:], in1=tb[:], op=bor)
        # v5 = (B3 >> 2) & 31
        nc.vector.tensor_scalar(
            out=vp[5], in0=bp[3], scalar1=2, scalar2=31, op0=sr, op1=band)
        # v6 = (B3 & 3) << 3 | B4 >> 5
        nc.vector.tensor_scalar(
            out=ta[:], in0=bp[3], scalar1=3, scalar2=3, op0=band, op1=sl)
        nc.vector.tensor_scalar(
            out=tb[:], in0=bp[4], scalar1=5, scalar2=None, op0=sr)
        nc.vector.tensor_tensor(out=vp[6], in0=ta[:], in1=tb[:], op=bor)
        # v7 = B4 & 31
        nc.vector.tensor_scalar(
            out=vp[7], in0=bp[4], scalar1=31, scalar2=None, op0=band)
        # dequant: labels p~ = (K/31)*v + K*eps; blank p~ = (K/255)*u + K*eps
        pf = dp.tile([BC, NV], F16, name="pf")
        nc.scalar.activation(
            out=pf[:, 0:PBASE], in_=uq[:],
            func=mybir.ActivationFunctionType.Copy,
            scale=KF / QLEV, bias=KF * EPS)
        nc.scalar.activation(
            out=pf[:, PBASE:NV], in_=plq[:, PB5:PB5 + T],
            func=mybir.ActivationFunctionType.Copy,
            scale=KF / QBL, bias=KF * EPS)
        # skip flags: exact 0.0 / 1.0
        skt = dp.tile([BC, L], F32, name="skt")
        nc.scalar.activation(
            out=skt[:], in_=plq[:, SKBASE:PACK],
            func=mybir.ActivationFunctionType.Copy, scale=1.0)
        pb = pf[:, PBASE:PBASE + T]

        # ---- DP over 65 lane pairs, packed coordinates p = t - j ----
        zz = dp.tile([BC, WO], F32, name="zz")
        d1e = dp.tile([BC, WE], F32, name="d1e")
        uu = dp.tile([BC, WO], F32, name="uu")
        d1o = dp.tile([BC, WO], F32, name="d1o")
        eex = dp.tile([BC, WO], F32, name="eex")   # col 0 stays 0 = E[j,p-1] pad
        oa = dp.tile([BC, WO], F32, name="oa")
        ob = dp.tile([BC, WO], F32, name="ob")
        nc.vector.memset(zz[:], 0.0)
        nc.vector.memset(eex[:], 0.0)

        mlt, pls = mybir.AluOpType.mult, mybir.AluOpType.add

        o_prev = zz
        for j in range(NL):
            pbj = pb[:, j:j + WE]
            if j == 0:
                nc.vector.tensor_tensor_scan(
                    eex[:, 1:WO], pbj, zz[:, 0:WE], 1.0, mlt, pls)
            else:
                nc.vector.tensor_tensor(
                    out=d1e[:], in0=pbj, in1=o_prev[:, 0:WE], op=mlt)
                nc.vector.tensor_tensor_scan(
                    eex[:, 1:WO], pbj, d1e[:], 0.0, mlt, pls)
            if j < L:
                o_cur = oa if (j % 2 == 0) else ob
                plj = pf[:, j * WO:(j + 1) * WO]
                nc.vector.scalar_tensor_tensor(
                    out=uu[:], in0=o_prev[:],
                    scalar=skt[:, j:j + 1], in1=eex[:],
                    op0=mlt, op1=pls)
                nc.vector.tensor_tensor(
                    out=d1o[:], in0=plj, in1=uu[:], op=mlt)
                nc.vector.tensor_tensor_scan(
                    o_cur[:], plj, d1o[:],
                    1.0 if j == 0 else 0.0, mlt, pls)
                o_prev = o_cur

        # loss = -(ln(E[64, T-1] + O[63, T-1]) - T*ln K)
        fin = dp.tile([BC, 1], F32, name="fin")
        lg = dp.tile([BC, 1], F32, name="lg")
        lo = dp.tile([BC, 1], F32, name="lo")
        nc.vector.tensor_tensor(
            out=fin[:], in0=eex[:, WO - 1:WO], in1=o_prev[:, WO - 1:WO],
            op=pls)
        nc.scalar.activation(
            out=lg[:], in_=fin[:], func=mybir.ActivationFunctionType.Ln)
        nc.vector.tensor_scalar(
            out=lo[:], in0=lg[:], scalar1=-1.0, scalar2=float(T) * CB_EFF,
            op0=mlt, op1=pls)
        nc.sync.dma_start(out=loss[:], in_=lo[:])


_CACHED_NC = None


def _build():
    global _CACHED_NC
    if _CACHED_NC is not None:
        return _CACHED_NC
    nc = bacc.Bacc("TRN2", target_bir_lowering=False, debug=False)
    plp_d = nc.dram_tensor("pl", [BC, PACK], U8, kind="ExternalInput")
    loss = nc.dram_tensor("loss", [BC, 1], F32, kind="ExternalOutput")
    with tile.TileContext(nc) as tc:
        _emit(nc, tc, plp_d, loss)
    nc.compile()
    _CACHED_NC = nc
    return nc


class _CachedSpmdRunner:
    """One-time-built jitted SPMD dispatch for a compiled Bass module.

    Mirrors what bass_utils.run_bass_kernel_spmd does under axon
    (shard_map over the _bass_exec_p custom-call on jax.devices()[:n]),
    but keeps the jitted callable so warm calls skip re-trace/re-compile
    and the remote keeps the loaded executable.
    """

    def __init__(self, nc, n_cores):
        import jax
        from jax.sharding import Mesh, PartitionSpec
        try:
            from jax.experimental.shard_map import shard_map
        except ImportError:  # newer jax
            from jax import shard_map
        from concourse.bass2jax import (
            install_neuronx_cc_hook, _bass_exec_p, partition_id_tensor)

        install_neuronx_cc_hook()
        self.n_cores = n_cores
        partition_name = (nc.partition_id_tensor.name
                          if nc.partition_id_tensor else None)
        in_names, out_names, out_avals, zero_outs = [], [], [], []
        for alloc in nc.m.functions[0].allocations:
            if not isinstance(alloc, mybir.MemoryLocationSet):
                continue
            name = alloc.memorylocations[0].name
            if alloc.kind == "ExternalInput":
                if name != partition_name:
                    in_names.append(name)
            elif alloc.kind == "ExternalOutput":
                shape = tuple(alloc.tensor_shape)
                dtype = mybir.dt.np(alloc.dtype)
                out_avals.append(jax.core.ShapedArray(shape, dtype))
                out_names.append(name)
                zero_outs.append(np.zeros(shape, dtype))
        self.dbg_name = None
        if nc.dbg_addr is not None:
            if nc.dbg_callbacks:
                raise RuntimeError("dbg_callbacks unsupported in this runner")
            self.dbg_name = nc.dbg_addr.name
            if self.dbg_name in in_names:
                in_names.remove(self.dbg_name)
            in_names.append(self.dbg_name)
        self.in_names = in_names
        self.out_names = out_names
        self.zero_outs = zero_outs
        n_params = len(in_names)
        n_outs = len(out_avals)
        all_in_names = in_names + out_names + (
            [partition_name] if partition_name else [])

        def _body(*args):
            operands = list(args)
            if partition_name is not None:
                operands.append(partition_id_tensor())
            outs = _bass_exec_p.bind(
                *operands,
                out_avals=tuple(out_avals),
                in_names=tuple(all_in_names),
                out_names=tuple(out_names),
                lowering_input_output_aliases=(),
                sim_require_finite=True,
                sim_require_nnan=True,
                nc=nc,
            )
            return tuple(outs)

        devices = jax.devices()[:n_cores]
        assert len(devices) == n_cores
        mesh = Mesh(np.asarray(devices), ("core",))
        in_specs = (PartitionSpec("core"),) * (n_params + n_outs)
        out_specs = (PartitionSpec("core"),) * n_outs
        donate = tuple(range(n_params, n_params + n_outs))
        self.fn = jax.jit(
            shard_map(_body, mesh=mesh, in_specs=in_specs,
                      out_specs=out_specs, check_rep=False),
            donate_argnums=donate, keep_unused=True,
        )

    def run(self, in_map):
        """in_map: full (n_cores*per_core_rows, ...) arrays keyed by name."""
        ins = []
        for name in self.in_names:
            if name == self.dbg_name:
                ins.append(np.zeros((self.n_cores, 2), np.uint32))
            else:
                ins.append(np.ascontiguousarray(in_map[name]))
        zeros = [np.zeros((self.n_cores * z.shape[0], *z.shape[1:]), z.dtype)
                 for z in self.zero_outs]
        out_arrs = self.fn(*ins, *zeros)
        return {name: np.asarray(a)
                for name, a in zip(self.out_names, out_arrs)}


_RUNNER = None


def _get_runner():
    global _RUNNER
    if _RUNNER is None:
        _RUNNER = _CachedSpmdRunner(_build(), NCORES)
    return _RUNNER


_NUMBA_FN = None
_NUMBA_TRIED = False


def _get_numba_fn():
    """Fused gather+quantize+pack: one pass over y_pred, no f32
    intermediate (the numpy path writes+rereads a 33 MiB temp).  Iterates
    (t, j) so reads stream row-wise and the ~64 open output lines per
    example stay cache-resident.  Falls back to numpy if numba is absent.
    """
    global _NUMBA_FN, _NUMBA_TRIED
    if not _NUMBA_TRIED:
        _NUMBA_TRIED = True
        try:
            import numba

            # literals match module constants: T=512, WO=449, WE window
            # arithmetic (448 = WO-1), L-1=63, PBASE=28736, NE=3592,
            # PB5=17960, blank col 255, label levels 31, blank levels 255
            @numba.njit(nogil=True, cache=True)
            def gather_pack(ypf, cols, plp, b0, b1):
                V = np.empty(28736, np.uint8)
                for b in range(b0, b1):
                    for t in range(512):
                        row = ypf[b, t]
                        jlo = t - 448 if t > 448 else 0
                        jhi = t if t < 63 else 63
                        for j in range(jlo, jhi + 1):
                            V[j * 449 + (t - j)] = np.uint8(
                                row[cols[b, j]] * np.float32(31.0)
                                + np.float32(0.5))
                        plp[b, 17960 + t] = np.uint8(
                            row[255] * np.float32(255.0) + np.float32(0.5))
                    for g in range(3592):
                        v0 = V[g]
                        v1 = V[3592 + g]
                        v2 = V[2 * 3592 + g]
                        v3 = V[3 * 3592 + g]
                        v4 = V[4 * 3592 + g]
                        v5 = V[5 * 3592 + g]
                        v6 = V[6 * 3592 + g]
                        v7 = V[7 * 3592 + g]
                        plp[b, g] = np.uint8((v0 << 3) | (v1 >> 2))
                        plp[b, 3592 + g] = np.uint8(
                            ((v1 & 3) << 6) | (v2 << 1) | (v3 >> 4))
                        plp[b, 2 * 3592 + g] = np.uint8(
                            ((v3 & 15) << 4) | (v4 >> 1))
                        plp[b, 3 * 3592 + g] = np.uint8(
                            ((v4 & 1) << 7) | (v5 << 2) | (v6 >> 3))
                        plp[b, 4 * 3592 + g] = np.uint8(((v6 & 7) << 5) | v7)

            _NUMBA_FN = gather_pack
        except Exception:
            _NUMBA_FN = None
    return _NUMBA_FN


def _host_prep(y_true, y_pred):
    lab = np.asarray(y_true).astype(np.int32)
    ypf = np.asarray(y_pred)
    if ypf.dtype != np.float32 or not ypf.flags.c_contiguous:
        ypf = np.ascontiguousarray(ypf, dtype=np.float32)
    cols = np.concatenate(
        [lab, np.full((B, 1), C - 1, np.int32)], axis=1)        # [B, NL]
    # uint8 linear quantization of y (dequantized on device as
    # (K/255)*u + K*eps); verified max rel err ~1.7e-3 on the loss.
    plp = np.empty((B, PACK), np.uint8)
    from concurrent.futures import ThreadPoolExecutor
    nth = 8
    step = (B + nth - 1) // nth
    nfn = _get_numba_fn()
    if nfn is not None:
        with ThreadPoolExecutor(nth) as ex:
            list(ex.map(lambda r: nfn(ypf, cols, plp, r[0], r[1]),
                        [(i, min(i + step, B)) for i in range(0, B, step)]))
    else:
        def _prep_slice(s):
            g = np.take_along_axis(ypf[s], cols[s][:, None, :], axis=2)
            nb = g.shape[0]
            # per-example transpose+quantize: the [65, 512] block stays
            # L2-resident, ~25% faster than transposing the whole slice
            q = np.empty((NL, T), np.uint8)
            v = np.lib.stride_tricks.as_strided(
                q, shape=(L, WO), strides=(T + 1, 1))
            V = np.empty(PBASE, np.uint8)
            qb = np.empty((NL, T), np.uint8)
            vb = np.lib.stride_tricks.as_strided(
                qb, shape=(L, WO), strides=(T + 1, 1))
            for i in range(nb):
                np.copyto(q, g[i].T * np.float32(QLEV) + np.float32(0.5),
                          casting="unsafe")
                np.copyto(qb, g[i].T * np.float32(QBL) + np.float32(0.5),
                          casting="unsafe")
                # packed label windows: lane j = q[j, j:j+WO]
                V[:] = v.reshape(PBASE)
                w = [V[k * NE:(k + 1) * NE] for k in range(8)]
                r = plp[s.start + i]
                r[0:NE] = (w[0] << 3) | (w[1] >> 2)
                r[NE:2 * NE] = ((w[1] & 3) << 6) | (w[2] << 1) | (w[3] >> 4)
                r[2 * NE:3 * NE] = ((w[3] & 15) << 4) | (w[4] >> 1)
                r[3 * NE:4 * NE] = ((w[4] & 1) << 7) | (w[5] << 2) | (w[6] >> 3)
                r[4 * NE:5 * NE] = ((w[6] & 7) << 5) | w[7]
                r[PB5:SKBASE] = qb[L]                           # 8-bit blank

        with ThreadPoolExecutor(nth) as ex:
            list(ex.map(_prep_slice,
                        [slice(i, min(i + step, B))
                         for i in range(0, B, step)]))
    plp[:, SKBASE] = 0
    plp[:, SKBASE + 1:PACK] = (lab[:, 1:] != lab[:, :-1])
    return plp


def kernel(y_true, y_pred):
    global _RUNNER
    plp = _host_prep(y_true, y_pred)
    out = None
    for attempt in range(2):
        try:
            res = _get_runner().run({"pl": plp})
            out = res["loss"]
            break
        except Exception:
            # e.g. transient NRT_EXEC_UNIT_UNRECOVERABLE: rebuild the
            # jitted dispatch (fresh executable load) and retry once.
            _RUNNER = None
    if out is None:
        # Fallback: the stock per-call SPMD dispatch path.
        from concourse.bass_utils import run_bass_kernel_spmd
        nc = _build()
        in_maps = [{"pl": plp[c * BC:(c + 1) * BC]} for c in range(NCORES)]
        r = run_bass_kernel_spmd(nc, in_maps, list(range(NCORES)))
        out = np.concatenate(
            [r.results[i]["loss"] for i in range(NCORES)], axis=0)
    return np.ascontiguousarray(out).astype(np.float32)
